# revision 4
# baseline (speedup 1.0000x reference)
"""Trainium2 Bass kernel v2 for a Mamba block (B=2, L=2048, d_model=1024,
d_inner=2048, d_state=16, d_conv=4, dt_rank=64), SPMD over 8 NeuronCores.

Sharding: 2 (batch) x 4 (d_inner shards of 512 channels), d-major layout.

Key points vs the v1 baseline:
- One pipelined loop over 8 sequence chunks of T=256; in_proj (PE), conv,
  x_dbl+AllReduce, delta, B/C broadcast, scan, out_proj overlap across chunks.
- float32r matmuls (4x PE throughput); out_proj in fp16.
- fp16 elementwise scan pipeline (dA/dBu/scan/C-mult/tree-reduce); the scan
  recurrence accumulates fp32 internally.
- silu computed as (tanh(x/2)+1)*x = 2*silu(x); the factors of 2 are folded
  into host-prescaled W_x (*0.5) and W_out (*0.25). This keeps every
  activation except the 4 delta-Ln ops per chunk in one act table
  (Tanh/Exp/Copy), avoiding table-reload thrash.
- y = sum_n(s*C) as an in-place halving tree of fp16 adds.
"""
import os
import sys
from contextlib import ExitStack

import numpy as np
import ml_dtypes

for _p in ("/opt/trn_rl_repo", "/root/.axon_site/_ro/trn_rl_repo"):
    if os.path.isdir(_p) and _p not in sys.path:
        sys.path.insert(0, _p)

import concourse.bass as bass
import concourse.mybir as mybir
import concourse.tile as tile
from concourse import bacc
from concourse.bass_utils import run_bass_kernel_spmd

F32 = mybir.dt.float32
F32R = mybir.dt.float32r
F16 = mybir.dt.float16
BF16 = mybir.dt.bfloat16
AF = mybir.ActivationFunctionType
OP = mybir.AluOpType

DM, DI, DS, DC, DR = 1024, 2048, 16, 4, 64
B, L = 2, 2048
NSH = 4            # d_inner shards per batch
DL = DI // NSH     # 512 channels per core
KT = DL // 128     # 4 partition tiles of channels
T = 256            # sequence chunk
NCH = L // T       # 8 chunks

CFG = {
    "conv": "v",     # conv chain: g(pool TT) / v(dve STT)
    "scan": "g",     # scan engine per k (string cycled to len KT)
    "dbu": "vvgg",   # dBu engine per k
    "cmul": "v",     # C-mult engine
    "dA": "a",       # per-k: a(exps on Act) / p(doubling on DVE)
    "bcopy": "a",    # psum->sbuf copy engine for broadcast B (g invalid: no PSUM on gpsimd)
    "ccopy": "a",    # psum->sbuf copy engine for broadcast C
    "zt": "v",       # z silu-mult engine
    "dx": "g",       # dx mult engine
    "gate": "g",     # gating mult engine
    "tree": "v",     # reduction tree engine
    "fold": "g",     # state-fold engine
    "otcopy": "a",   # out_proj psum->sbuf copy engine
    "fp32r": True,
    "elem16": True,  # fp16 elementwise pipeline
}


def _eng(nc, ch):
    return {"v": nc.vector, "g": nc.gpsimd, "a": nc.scalar}[ch]


def _copy(nc, ch):
    return nc.scalar.copy if ch == "a" else _eng(nc, ch).tensor_copy


def build_program(n_reps: int = 1, use_collective: bool = True, cfg=None):
    cfg = dict(CFG, **(cfg or {}))
    EF = F16 if cfg["elem16"] else F32

    def mmcast(ap):
        return ap.bitcast(F32R) if cfg["fp32r"] else ap

    nc = bacc.Bacc("TRN2", target_bir_lowering=False)
    hsT = nc.declare_dram_parameter("hsT", [DM, L], BF16, isOutput=False)
    wix = nc.declare_dram_parameter("wix", [DM, DL], BF16, isOutput=False)
    wiz = nc.declare_dram_parameter("wiz", [DM, DL], BF16, isOutput=False)
    wc = nc.declare_dram_parameter("wc", [DL, DC], F32, isOutput=False)
    bcv = nc.declare_dram_parameter("bcv", [DL, 1], F32, isOutput=False)
    wx = nc.declare_dram_parameter("wx", [DL, 96], F32, isOutput=False)
    wdt = nc.declare_dram_parameter("wdt", [DR, DL], F16, isOutput=False)
    bdt = nc.declare_dram_parameter("bdt", [DL, 1], F32, isOutput=False)
    asc = nc.declare_dram_parameter("asc", [DL, DS], F32, isOutput=False)
    dpar = nc.declare_dram_parameter("dpar", [DL, 1], F32, isOutput=False)
    wout = nc.declare_dram_parameter("wout", [DL, DM], F16, isOutput=False)
    sel = nc.declare_dram_parameter("sel", [2 * DS, 2 * DS * 128], F16,
                                    isOutput=False)
    outp = nc.declare_dram_parameter("outp", [L, DM], F16, isOutput=True)

    with tile.TileContext(nc) as tc, ExitStack() as ctx:
        def emit_once():
            dram = ctx.enter_context(tc.tile_pool(name="dram", bufs=1, space="DRAM"))
            xd_bounce = [dram.tile([96, T], F16, name=f"xdb{c}") for c in range(NCH)]
            xd_red = [dram.tile([96, T], F16, name=f"xdr{c}") for c in range(NCH)]

            consts = ctx.enter_context(tc.tile_pool(name="consts", bufs=1))
            wc_t = consts.tile([128, DC * KT], F32, tag="wc")
            bcv_t = consts.tile([128, KT], F32, tag="bcv")
            hbcv_t = consts.tile([128, KT], F32, tag="hbcv")
            bdt_t = consts.tile([128, KT], F32, tag="bdt")
            asc_t = consts.tile([128, DS * KT], F32, tag="asc")
            dpar_t = consts.tile([128, KT], F32, tag="dpar")
            for k in range(KT):
                ksl = slice(128 * k, 128 * (k + 1))
                nc.scalar.dma_start(wc_t[:, DC * k:DC * (k + 1)], wc[ksl, :])
                nc.scalar.dma_start(bcv_t[:, k:k + 1], bcv[ksl, :])
                nc.scalar.dma_start(bdt_t[:, k:k + 1], bdt[ksl, :])
                nc.scalar.dma_start(asc_t[:, DS * k:DS * (k + 1)], asc[ksl, :])
                nc.scalar.dma_start(dpar_t[:, k:k + 1], dpar[ksl, :])
            nc.vector.tensor_scalar(hbcv_t[:], bcv_t[:], 0.5, None, OP.mult)
            # selection matrix for B/C partition-broadcast matmuls
            sel16 = consts.tile([2 * DS, 2 * DS * 128], F16, tag="sel")
            nc.scalar.dma_start(sel16[:], sel[:])

            persist = ctx.enter_context(tc.tile_pool(name="persist", bufs=1))
            wix_t = [persist.tile([128, DL], BF16, tag=f"wix{kk}", name=f"wix{kk}")
                     for kk in range(8)]
            wiz_t = [persist.tile([128, DL], BF16, tag=f"wiz{kk}", name=f"wiz{kk}")
                     for kk in range(8)]
            for kk in range(8):
                nc.scalar.dma_start(wix_t[kk][:], wix[128 * kk:128 * (kk + 1), :])
                nc.scalar.dma_start(wiz_t[kk][:], wiz[128 * kk:128 * (kk + 1), :])
            wx_t = [persist.tile([128, 96], F32, tag=f"wx{k}", name=f"wx{k}")
                    for k in range(KT)]
            wxr_t = [persist.tile([128, 96], F32R, tag=f"wxr{k}", name=f"wxr{k}")
                     for k in range(KT)]
            wout_t = [persist.tile([128, DM], F16, tag=f"wo{k}", name=f"wo{k}")
                      for k in range(KT)]
            wdt_t = persist.tile([128, DL], F16, tag="wdt")
            nc.scalar.dma_start(wdt_t[0:DR, :], wdt[:])
            for k in range(KT):
                ksl = slice(128 * k, 128 * (k + 1))
                nc.scalar.dma_start(wx_t[k][:], wx[ksl, :])
                nc.scalar.copy(wxr_t[k][:], wx_t[k][:])
                nc.scalar.dma_start(wout_t[k][:], wout[ksl, :])

            xpad = [persist.tile([128, L + 3], F16, tag=f"xp{k}", name=f"xp{k}")
                    for k in range(KT)]
            for k in range(KT):
                nc.vector.memset(xpad[k][:, 0:1], 0.0)
                nc.vector.memset(xpad[k][:, L + 1:L + 3], 0.0)
            state_t = persist.tile([128, DS * KT], EF, tag="state")
            nc.vector.memset(state_t[:], 0.0)

            hs_pool = ctx.enter_context(tc.tile_pool(name="hs", bufs=3))
            ps_in = ctx.enter_context(tc.tile_pool(name="psin", bufs=1, space="PSUM"))
            ps_x = ctx.enter_context(tc.tile_pool(name="psx", bufs=2, space="PSUM"))
            ps_bc = ctx.enter_context(tc.tile_pool(name="psbc", bufs=2, space="PSUM"))
            ps_out = ctx.enter_context(tc.tile_pool(name="psout", bufs=2, space="PSUM"))
            cvp = ctx.enter_context(tc.tile_pool(name="cv", bufs=2))
            z_p = ctx.enter_context(tc.tile_pool(name="zp", bufs=3))
            xs_p = ctx.enter_context(tc.tile_pool(name="xs", bufs=3))
            xd_p = ctx.enter_context(tc.tile_pool(name="xd", bufs=2))
            bct_p = ctx.enter_context(tc.tile_pool(name="bct", bufs=2))
            del_p = ctx.enter_context(tc.tile_pool(name="del", bufs=2))
            bc_p = ctx.enter_context(tc.tile_pool(name="bc", bufs=2))
            mid = ctx.enter_context(tc.tile_pool(name="mid", bufs=2))
            s_pool = ctx.enter_context(tc.tile_pool(name="sp", bufs=2))
            yg_p = ctx.enter_context(tc.tile_pool(name="yg", bufs=2))
            outc = ctx.enter_context(tc.tile_pool(name="outc", bufs=1))

            ceng = _eng(nc, cfg["conv"])
            meng = _eng(nc, cfg["cmul"])
            teng = _eng(nc, cfg["tree"])

            z_tiles = {}

            def in_proj(c):
                lsl = slice(T * c, T * (c + 1))
                z_tiles[c] = [None] * KT
                for half, wt in ((0, wix_t), (1, wiz_t)):
                    pi = [ps_in.tile([128, 2 * T], F32, tag=f"pi{j}", name=f"pi{j}")
                          for j in range(2)]
                    for kk in range(8):
                        h = hs_pool.tile([128, T], BF16, tag="hs")
                        nc.sync.dma_start(h[:], hsT[128 * kk:128 * (kk + 1), lsl])
                        for k in range(KT):
                            # 2 k-tiles share one psum bank as ONE accumulation
                            # group: start zeroes the whole bank, so only the
                            # first matmul starts and only the last stops
                            reg = pi[k // 2][:, T * (k % 2):T * (k % 2 + 1)]
                            nc.tensor.matmul(reg,
                                             wt[kk][:, 128 * k:128 * (k + 1)],
                                             h[:],
                                             start=(kk == 0 and k % 2 == 0),
                                             stop=(kk == 7 and k % 2 == 1))
                    for k in range(KT):
                        reg = pi[k // 2][:, T * (k % 2):T * (k % 2 + 1)]
                        if half == 0:
                            nc.scalar.copy(
                                xpad[k][:, 1 + T * c:1 + T * (c + 1)], reg)
                        else:
                            # zt = (tanh(z/2)+1)*z = 2*silu(z)
                            th = cvp.tile([128, T], F32, tag="zth")
                            nc.scalar.activation(th[:], reg, AF.Tanh, scale=0.5)
                            zt = z_p.tile([128, T], F16, tag=f"z{k}",
                                          name=f"z{k}")
                            z_tiles[c][k] = zt
                            _eng(nc, cfg["zt"]).scalar_tensor_tensor(
                                zt[:], th[:], 1.0, reg, OP.add, OP.mult)

            xs_tiles = {}

            def head(c):
                # ---- conv; xs = 2*silu(conv+bcv) (wx prescaled by 0.5) ----
                xs_c = xs_tiles[c] = []
                for k in range(KT):
                    base = T * c

                    def wcb(i):
                        col = wc_t[:, DC * k + i:DC * k + i + 1]
                        return col.to_broadcast([128, T])

                    if cfg["conv"] == "v":
                        t0 = cvp.tile([128, T], F32, tag="cv")
                        nc.vector.tensor_scalar(
                            t0[:], xpad[k][:, base:base + T],
                            wc_t[:, DC * k:DC * k + 1], None, OP.mult)
                        t3 = t0
                        for i in (1, 2, 3):
                            ti = cvp.tile([128, T], F32, tag="cv")
                            nc.vector.scalar_tensor_tensor(
                                ti[:], xpad[k][:, base + i:base + i + T],
                                wc_t[:, DC * k + i:DC * k + i + 1], t3[:],
                                OP.mult, OP.add)
                            t3 = ti
                    else:
                        # gpsimd has no tensor_scalar/STT: TT with free-dim
                        # broadcast weights, pairwise sum
                        m0 = cvp.tile([128, T], F32, tag="cv")
                        nc.gpsimd.tensor_tensor(m0[:], xpad[k][:, base:base + T],
                                                wcb(0), OP.mult)
                        m1 = cvp.tile([128, T], F32, tag="cv")
                        nc.gpsimd.tensor_tensor(
                            m1[:], xpad[k][:, base + 1:base + 1 + T], wcb(1),
                            OP.mult)
                        nc.gpsimd.tensor_tensor(m0[:], m0[:], m1[:], OP.add)
                        m2 = cvp.tile([128, T], F32, tag="cv2")
                        nc.gpsimd.tensor_tensor(
                            m2[:], xpad[k][:, base + 2:base + 2 + T], wcb(2),
                            OP.mult)
                        m3 = cvp.tile([128, T], F32, tag="cv2")
                        nc.gpsimd.tensor_tensor(
                            m3[:], xpad[k][:, base + 3:base + 3 + T], wcb(3),
                            OP.mult)
                        nc.gpsimd.tensor_tensor(m2[:], m2[:], m3[:], OP.add)
                        t3 = cvp.tile([128, T], F32, tag="cv3")
                        nc.gpsimd.tensor_tensor(t3[:], m0[:], m2[:], OP.add)
                    th = cvp.tile([128, T], F32, tag="cth")
                    nc.scalar.activation(th[:], t3[:], AF.Tanh, scale=0.5,
                                         bias=hbcv_t[:, k:k + 1])
                    xb = cvp.tile([128, T], F32, tag="xb")
                    bcvb = bcv_t[:, k:k + 1].to_broadcast([128, T])
                    ceng.tensor_tensor(xb[:], t3[:], bcvb, OP.add)
                    xk = xs_p.tile([128, T], F32R, tag=f"xs{k}", name=f"xs{k}")
                    if cfg["conv"] == "v":
                        nc.vector.scalar_tensor_tensor(xk[:], th[:], 1.0, xb[:],
                                                       OP.add, OP.mult)
                    else:
                        xm = cvp.tile([128, T], F32, tag="xm")
                        nc.gpsimd.tensor_tensor(xm[:], xb[:], th[:], OP.mult)
                        nc.vector.tensor_tensor(xk[:], xm[:], xb[:], OP.add)
                    xs_c.append(xk)

                # ---- x_dbl partial + AllReduce ----
                pxd = ps_x.tile([128, T], F32, tag="pxd", name="pxd")
                for k in range(KT):
                    nc.tensor.matmul(pxd[0:96, :], wxr_t[k][:], xs_c[k][:],
                                     start=(k == 0), stop=(k == KT - 1))
                xt = xd_p.tile([96, T], F16, tag="xt")
                nc.scalar.copy(xt[:], pxd[0:96, :])
                nc.sync.dma_start(xd_bounce[c][:], xt[:])
                if use_collective:
                    nc.gpsimd.collective_compute(
                        "AllReduce", OP.add,
                        replica_groups=[[0, 1, 2, 3], [4, 5, 6, 7]],
                        ins=[xd_bounce[c].opt()], outs=[xd_red[c].opt()])
                else:
                    nc.sync.dma_start(xd_red[c][:], xd_bounce[c][:])

            def stail(c):
                lsl = slice(T * c, T * (c + 1))
                xs_c = xs_tiles[c]
                xdb = xd_p.tile([64, T], F16, tag="xdb")
                nc.sync.dma_start(xdb[:], xd_red[c][0:DR, :])
                bct16 = bct_p.tile([2 * DS, T], F16, tag="bct16")
                nc.sync.dma_start(bct16[:], xd_red[c][DR:DR + 2 * DS, :])

                # ---- delta = softplus(dt): blocked Exp x4 then Ln x4 ----
                pdts = []
                for kp in range(2):
                    pdt = ps_bc.tile([128, 2 * T], F32, tag="pb")
                    for j in range(2):
                        k = 2 * kp + j
                        nc.tensor.matmul(
                            pdt[:, T * j:T * (j + 1)],
                            wdt_t[0:DR, 128 * k:128 * (k + 1)],
                            xdb[:], start=(j == 0), stop=(j == 1))
                    pdts.append(pdt)
                deltas = []
                for k in range(KT):
                    dlt = del_p.tile([128, T], F32, tag=f"d{k}", name=f"d{k}")
                    nc.scalar.activation(dlt[:], pdts[k // 2][:, T * (k % 2):
                                                             T * (k % 2 + 1)],
                                         AF.Exp, bias=bdt_t[:, k:k + 1])
                    deltas.append(dlt)
                for k in range(KT):
                    nc.scalar.activation(deltas[k][:], deltas[k][:], AF.Ln,
                                         bias=1.0)

                # ---- B/C partition broadcast ----
                b_bc = bc_p.tile([128, DS * T], EF, tag="b_bc")
                c_bc = bc_p.tile([128, DS * T], EF, tag="c_bc")
                for half, dst, cpc in ((0, b_bc, cfg["bcopy"]),
                                       (1, c_bc, cfg["ccopy"])):
                    pcopy = _copy(nc, cpc)
                    for g in range(DS // 2):
                        pb = ps_bc.tile([128, 2 * T], F32, tag="pb")
                        for j in range(2):
                            n = 16 * half + 2 * g + j
                            nc.tensor.matmul(pb[:, T * j:T * (j + 1)],
                                             sel16[:, 128 * n:128 * (n + 1)],
                                             bct16[:], start=(j == 0),
                                             stop=(j == 1))
                        pcopy(dst[:, 2 * T * g:2 * T * (g + 1)], pb[:])

                # ---- per-k scan blocks ----
                g_list = []
                for k in range(KT):
                    nsl = slice(DS * k, DS * (k + 1))
                    d_ch = deltas[k]
                    dx = mid.tile([128, T], EF, tag="dx")
                    _eng(nc, cfg["dx"]).tensor_tensor(
                        dx[:], d_ch[:], xs_c[k][:].bitcast(F32), OP.mult)
                    dA = mid.tile([128, DS * T], EF, tag="dA")
                    mode = {"a": "act", "p": "pow"}[(cfg["dA"] * KT)[k]]
                    if mode == "act":
                        for n in range(DS):
                            nc.scalar.activation(
                                dA[:, T * n:T * (n + 1)], d_ch[:], AF.Exp,
                                scale=asc_t[:, DS * k + n:DS * k + n + 1])
                    else:
                        nc.scalar.activation(dA[:, 0:T], d_ch[:], AF.Exp,
                                             scale=-1.0)
                        nc.vector.tensor_tensor(dA[:, T:2 * T], dA[:, 0:T],
                                                dA[:, 0:T], OP.mult)
                        for (lo, n_seg) in ((2, 2), (4, 4), (8, 8)):
                            src = dA[:, (lo - 1) * T:lo * T]
                            nc.vector.tensor_tensor(
                                dA[:, lo * T:2 * lo * T].rearrange(
                                    "p (n l) -> p n l", n=n_seg),
                                dA[:, 0:lo * T].rearrange(
                                    "p (n l) -> p n l", n=n_seg),
                                src[:, None, :].to_broadcast([128, n_seg, T]),
                                OP.mult)
                    dBu = mid.tile([128, DS * T], EF, tag="dBu")
                    deng = _eng(nc, (cfg["dbu"] * KT)[k])
                    deng.tensor_tensor(
                        dBu[:].rearrange("p (n l) -> p n l", n=DS),
                        dx[:, None, :].to_broadcast([128, DS, T]),
                        b_bc[:].rearrange("p (n l) -> p n l", n=DS),
                        OP.mult)
                    # fold carried state into first column of each segment
                    feng = _eng(nc, cfg["fold"])
                    fx = mid.tile([128, DS], EF, tag="fx")
                    feng.tensor_tensor(fx[:], dA[:, 0:DS * T:T],
                                       state_t[:, nsl], OP.mult)
                    feng.tensor_tensor(dBu[:, 0:DS * T:T],
                                       dBu[:, 0:DS * T:T], fx[:], OP.add)
                    nc.gpsimd.memset(dA[:, 0:DS * T:T], 0.0)
                    s_t = s_pool.tile([128, DS * T], EF, tag="s")
                    seng = nc.vector  # hw: TensorTensorScan only exists on DVE
                    seng.tensor_tensor_scan(s_t[:], dA[:], dBu[:], 0.0,
                                            OP.mult, OP.add)
                    seng.tensor_copy(state_t[:, nsl], s_t[:, T - 1:DS * T:T])
                    # y = sum_n s*C: in-place mult, then halving tree
                    meng.tensor_tensor(s_t[:], s_t[:], c_bc[:], OP.mult)
                    for w in (8, 4, 2):
                        teng.tensor_tensor(
                            s_t[:, 0:w * T], s_t[:, 0:w * T],
                            s_t[:, w * T:2 * w * T], OP.add)
                    y_r = yg_p.tile([128, T], F32, tag="yr")
                    teng.tensor_tensor(y_r[:], s_t[:, 0:T], s_t[:, T:2 * T],
                                       OP.add)
                    # skip + gate
                    nc.vector.scalar_tensor_tensor(
                        y_r[:], xs_c[k][:].bitcast(F32), dpar_t[:, k:k + 1],
                        y_r[:], OP.mult, OP.add)
                    g_t = yg_p.tile([128, T], F16, tag=f"g{k}", name=f"g{k}")
                    _eng(nc, cfg["gate"]).tensor_tensor(
                        g_t[:], y_r[:], z_tiles[c][k][:], OP.mult)
                    g_list.append(g_t)

                # ---- out_proj (wout prescaled by 0.25) ----
                ocopy = _copy(nc, cfg["otcopy"])
                for h in range(T // 128):
                    msl = slice(128 * h, 128 * (h + 1))
                    rsl = slice(T * c + 128 * h, T * c + 128 * (h + 1))
                    ot = outc.tile([128, DM], F16, tag="ot")
                    for col in range(2):
                        po = ps_out.tile([128, 512], F32, tag="po")
                        for k in range(KT):
                            nc.tensor.matmul(
                                po[:], g_list[k][:, msl],
                                wout_t[k][:, 512 * col:512 * (col + 1)],
                                start=(k == 0), stop=(k == KT - 1))
                        ocopy(ot[:, 512 * col:512 * (col + 1)], po[:])
                    nc.sync.dma_start(outp[rsl, :], ot[:])

            for c in range(NCH):
                in_proj(c)
                if c >= 1:
                    head(c - 1)
                if c >= 2:
                    stail(c - 2)
            head(NCH - 1)
            stail(NCH - 2)
            stail(NCH - 1)

        for _rep in range(n_reps):
            emit_once()
    nc.compile()
    return nc


def make_in_maps(inputs):
    hs = np.ascontiguousarray(np.asarray(inputs["hidden_states"], np.float32))
    W_in = np.asarray(inputs["W_in"], np.float32)
    W_conv = np.asarray(inputs["W_conv"], np.float32)
    b_conv = np.asarray(inputs["b_conv"], np.float32)
    W_x = np.asarray(inputs["W_x"], np.float32)
    W_dt = np.asarray(inputs["W_dt"], np.float32)
    b_dt = np.asarray(inputs["b_dt"], np.float32)
    A_log = np.asarray(inputs["A_log"], np.float32)
    D_param = np.asarray(inputs["D_param"], np.float32)
    W_out = np.asarray(inputs["W_out"], np.float32)
    A = -np.exp(A_log.astype(np.float64)).astype(np.float32)
    sel_mat = np.zeros((2 * DS, 2 * DS * 128), np.float16)
    for n in range(2 * DS):
        sel_mat[n, 128 * n:128 * (n + 1)] = 1.0

    in_maps = []
    for cid in range(8):
        b, s = cid // NSH, cid % NSH
        sh = slice(DL * s, DL * (s + 1))
        in_maps.append({
            "hsT": np.ascontiguousarray(hs[b].T).astype(ml_dtypes.bfloat16),
            "wix": np.ascontiguousarray(
                W_in[:, 2 * DL * s:2 * DL * (s + 1):2]).astype(ml_dtypes.bfloat16),
            "wiz": np.ascontiguousarray(
                W_in[:, 2 * DL * s + 1:2 * DL * (s + 1) + 1:2]).astype(
                    ml_dtypes.bfloat16),
            "wc": np.ascontiguousarray(W_conv[:, 0, sh].T),
            "bcv": np.ascontiguousarray(b_conv[sh].reshape(DL, 1)),
            # x is carried as 2*silu(.): fold 0.5 into W_x, 0.25 into W_out
            "wx": np.ascontiguousarray(0.5 * W_x[sh, :]),
            "wdt": np.ascontiguousarray(W_dt[:, sh].astype(np.float16)),
            "bdt": np.ascontiguousarray(b_dt[sh].reshape(DL, 1)),
            "asc": np.ascontiguousarray(A[sh, :]),
            "dpar": np.ascontiguousarray(D_param[sh].reshape(DL, 1)),
            "wout": np.ascontiguousarray(
                (0.25 * W_out[sh, :]).astype(np.float16)),
            "sel": sel_mat,
        })
    return in_maps


_NC_CACHE = None
_LAST_IN_MAPS = None


def kernel(**inputs) -> np.ndarray:
    global _NC_CACHE, _LAST_IN_MAPS
    in_maps = make_in_maps(inputs)
    _LAST_IN_MAPS = in_maps
    if _NC_CACHE is None:
        _NC_CACHE = build_program()
    res = run_bass_kernel_spmd(_NC_CACHE, in_maps, list(range(8)))
    out = np.zeros((B, L, DM), np.float32)
    for cid in range(8):
        out[cid // NSH] += res.results[cid]["outp"].astype(np.float32)
    return out


if __name__ == "__main__":
    rng = np.random.default_rng(0)
    dummy = {
        "hidden_states": rng.standard_normal((B, L, DM), dtype=np.float32),
        "W_in": rng.standard_normal((DM, 2 * DI), dtype=np.float32) * 0.03,
        "W_conv": rng.standard_normal((DC, 1, DI), dtype=np.float32) * 0.5,
        "b_conv": np.zeros((DI,), np.float32),
        "W_x": rng.standard_normal((DI, DR + 2 * DS), dtype=np.float32) * 0.02,
        "W_dt": rng.standard_normal((DR, DI), dtype=np.float32) * 0.12,
        "b_dt": rng.standard_normal((DI,), dtype=np.float32) * 0.01,
        "A_log": np.log(np.broadcast_to(np.arange(1, DS + 1, dtype=np.float32),
                                        (DI, DS))).copy(),
        "D_param": np.ones((DI,), np.float32),
        "W_out": rng.standard_normal((DI, DM), dtype=np.float32) * 0.03,
    }
    out = kernel(**dummy)
    print("out", out.shape, out.dtype, np.abs(out).max())


# revision 5
# speedup vs baseline: 1.8827x; 1.8827x over previous
"""Trainium2 Bass kernel v2 for a Mamba block (B=2, L=2048, d_model=1024,
d_inner=2048, d_state=16, d_conv=4, dt_rank=64), SPMD over 8 NeuronCores.

Sharding: 2 (batch) x 4 (d_inner shards of 512 channels), d-major layout.

Key points vs the v1 baseline:
- One pipelined loop over 8 sequence chunks of T=256; in_proj (PE), conv,
  x_dbl+AllReduce, delta, B/C broadcast, scan, out_proj overlap across chunks.
- float32r matmuls (4x PE throughput); out_proj in fp16.
- fp16 elementwise scan pipeline (dA/dBu/scan/C-mult/tree-reduce); the scan
  recurrence accumulates fp32 internally.
- silu computed as (tanh(x/2)+1)*x = 2*silu(x); the factors of 2 are folded
  into host-prescaled W_x (*0.5) and W_out (*0.25). This keeps every
  activation except the 4 delta-Ln ops per chunk in one act table
  (Tanh/Exp/Copy), avoiding table-reload thrash.
- y = sum_n(s*C) as an in-place halving tree of fp16 adds.
"""
import os
import sys
from contextlib import ExitStack

import numpy as np
import ml_dtypes

for _p in ("/opt/trn_rl_repo", "/root/.axon_site/_ro/trn_rl_repo"):
    if os.path.isdir(_p) and _p not in sys.path:
        sys.path.insert(0, _p)

import concourse.bass as bass
import concourse.mybir as mybir
import concourse.tile as tile
from concourse import bacc
from concourse.bass_utils import run_bass_kernel_spmd

F32 = mybir.dt.float32
F32R = mybir.dt.float32r
F16 = mybir.dt.float16
BF16 = mybir.dt.bfloat16
AF = mybir.ActivationFunctionType
OP = mybir.AluOpType

DM, DI, DS, DC, DR = 1024, 2048, 16, 4, 64
B, L = 2, 2048
NSH = 4            # d_inner shards per batch
DL = DI // NSH     # 512 channels per core
KT = DL // 128     # 4 partition tiles of channels
T = 256            # sequence chunk
NCH = L // T       # 8 chunks

CFG = {
    "conv": "v",     # conv chain: g(pool TT) / v(dve STT)
    "scan": "g",     # scan engine per k (string cycled to len KT)
    "dbu": "vvgg",   # dBu engine per k
    "cmul": "v",     # C-mult engine
    "dA": "a",       # per-k: a(exps on Act) / p(doubling on DVE)
    "bcopy": "a",    # psum->sbuf copy engine for broadcast B (g invalid: no PSUM on gpsimd)
    "ccopy": "a",    # psum->sbuf copy engine for broadcast C
    "zt": "v",       # z silu-mult engine
    "dx": "g",       # dx mult engine
    "gate": "g",     # gating mult engine
    "tree": "v",     # reduction tree engine
    "fold": "g",     # state-fold engine
    "otcopy": "a",   # out_proj psum->sbuf copy engine
    "fp32r": True,
    "elem16": True,  # fp16 elementwise pipeline
}


def _eng(nc, ch):
    return {"v": nc.vector, "g": nc.gpsimd, "a": nc.scalar}[ch]


def _copy(nc, ch):
    return nc.scalar.copy if ch == "a" else _eng(nc, ch).tensor_copy


def build_program(n_reps: int = 1, use_collective: bool = True, cfg=None):
    cfg = dict(CFG, **(cfg or {}))
    EF = F16 if cfg["elem16"] else F32

    def mmcast(ap):
        return ap.bitcast(F32R) if cfg["fp32r"] else ap

    nc = bacc.Bacc("TRN2", target_bir_lowering=False)
    hsT = nc.declare_dram_parameter("hsT", [DM, L], BF16, isOutput=False)
    wix = nc.declare_dram_parameter("wix", [DM, DL], BF16, isOutput=False)
    wiz = nc.declare_dram_parameter("wiz", [DM, DL], BF16, isOutput=False)
    wc = nc.declare_dram_parameter("wc", [DL, DC], F32, isOutput=False)
    bcv = nc.declare_dram_parameter("bcv", [DL, 1], F32, isOutput=False)
    wx = nc.declare_dram_parameter("wx", [DL, 96], F32, isOutput=False)
    wdt = nc.declare_dram_parameter("wdt", [DR, DL], F16, isOutput=False)
    bdt = nc.declare_dram_parameter("bdt", [DL, 1], F32, isOutput=False)
    asc = nc.declare_dram_parameter("asc", [DL, DS], F32, isOutput=False)
    dpar = nc.declare_dram_parameter("dpar", [DL, 1], F32, isOutput=False)
    wout = nc.declare_dram_parameter("wout", [DL, DM], F16, isOutput=False)
    sel = nc.declare_dram_parameter("sel", [2 * DS, 2 * DS * 128], F16,
                                    isOutput=False)
    outp = nc.declare_dram_parameter("outp", [L, DM], F16, isOutput=True)

    with tile.TileContext(nc) as tc:
        def emit_once(ctx):
            dram = ctx.enter_context(tc.tile_pool(name="dram", bufs=1, space="DRAM"))
            xd_bounce = [dram.tile([96, T], F16, name=f"xdb{c}") for c in range(NCH)]
            xd_red = [dram.tile([96, T], F16, name=f"xdr{c}") for c in range(NCH)]

            consts = ctx.enter_context(tc.tile_pool(name="consts", bufs=1))
            wc_t = consts.tile([128, DC * KT], F32, tag="wc")
            bcv_t = consts.tile([128, KT], F32, tag="bcv")
            hbcv_t = consts.tile([128, KT], F32, tag="hbcv")
            bdt_t = consts.tile([128, KT], F32, tag="bdt")
            asc_t = consts.tile([128, DS * KT], F32, tag="asc")
            dpar_t = consts.tile([128, KT], F32, tag="dpar")
            for k in range(KT):
                ksl = slice(128 * k, 128 * (k + 1))
                nc.scalar.dma_start(wc_t[:, DC * k:DC * (k + 1)], wc[ksl, :])
                nc.scalar.dma_start(bcv_t[:, k:k + 1], bcv[ksl, :])
                nc.scalar.dma_start(bdt_t[:, k:k + 1], bdt[ksl, :])
                nc.scalar.dma_start(asc_t[:, DS * k:DS * (k + 1)], asc[ksl, :])
                nc.scalar.dma_start(dpar_t[:, k:k + 1], dpar[ksl, :])
            nc.vector.tensor_scalar(hbcv_t[:], bcv_t[:], 0.5, None, OP.mult)
            # selection matrix for B/C partition-broadcast matmuls
            sel16 = consts.tile([2 * DS, 2 * DS * 128], F16, tag="sel")
            nc.scalar.dma_start(sel16[:], sel[:])

            persist = ctx.enter_context(tc.tile_pool(name="persist", bufs=1))
            wix_t = [persist.tile([128, DL], BF16, tag=f"wix{kk}", name=f"wix{kk}")
                     for kk in range(8)]
            wiz_t = [persist.tile([128, DL], BF16, tag=f"wiz{kk}", name=f"wiz{kk}")
                     for kk in range(8)]
            for kk in range(8):
                nc.scalar.dma_start(wix_t[kk][:], wix[128 * kk:128 * (kk + 1), :])
                nc.scalar.dma_start(wiz_t[kk][:], wiz[128 * kk:128 * (kk + 1), :])
            wx_t = [persist.tile([128, 96], F32, tag=f"wx{k}", name=f"wx{k}")
                    for k in range(KT)]
            wxr_t = [persist.tile([128, 96], F32R, tag=f"wxr{k}", name=f"wxr{k}")
                     for k in range(KT)]
            wout_t = [persist.tile([128, DM], F16, tag=f"wo{k}", name=f"wo{k}")
                      for k in range(KT)]
            wdt_t = persist.tile([128, DL], F16, tag="wdt")
            nc.scalar.dma_start(wdt_t[0:DR, :], wdt[:])
            for k in range(KT):
                ksl = slice(128 * k, 128 * (k + 1))
                nc.scalar.dma_start(wx_t[k][:], wx[ksl, :])
                nc.scalar.copy(wxr_t[k][:], wx_t[k][:])
                nc.scalar.dma_start(wout_t[k][:], wout[ksl, :])

            xpad = [persist.tile([128, L + 3], F16, tag=f"xp{k}", name=f"xp{k}")
                    for k in range(KT)]
            for k in range(KT):
                nc.vector.memset(xpad[k][:, 0:1], 0.0)
                nc.vector.memset(xpad[k][:, L + 1:L + 3], 0.0)
            state_t = persist.tile([128, DS * KT], EF, tag="state")
            nc.vector.memset(state_t[:], 0.0)

            hs_pool = ctx.enter_context(tc.tile_pool(name="hs", bufs=3))
            ps_in = ctx.enter_context(tc.tile_pool(name="psin", bufs=1, space="PSUM"))
            ps_x = ctx.enter_context(tc.tile_pool(name="psx", bufs=2, space="PSUM"))
            ps_bc = ctx.enter_context(tc.tile_pool(name="psbc", bufs=2, space="PSUM"))
            ps_out = ctx.enter_context(tc.tile_pool(name="psout", bufs=2, space="PSUM"))
            cvp = ctx.enter_context(tc.tile_pool(name="cv", bufs=2))
            z_p = ctx.enter_context(tc.tile_pool(name="zp", bufs=3))
            xs_p = ctx.enter_context(tc.tile_pool(name="xs", bufs=3))
            xd_p = ctx.enter_context(tc.tile_pool(name="xd", bufs=2))
            bct_p = ctx.enter_context(tc.tile_pool(name="bct", bufs=2))
            del_p = ctx.enter_context(tc.tile_pool(name="del", bufs=2))
            bc_p = ctx.enter_context(tc.tile_pool(name="bc", bufs=2))
            mid = ctx.enter_context(tc.tile_pool(name="mid", bufs=2))
            s_pool = ctx.enter_context(tc.tile_pool(name="sp", bufs=2))
            yg_p = ctx.enter_context(tc.tile_pool(name="yg", bufs=2))
            outc = ctx.enter_context(tc.tile_pool(name="outc", bufs=1))

            ceng = _eng(nc, cfg["conv"])
            meng = _eng(nc, cfg["cmul"])
            teng = _eng(nc, cfg["tree"])

            z_tiles = {}

            def in_proj(c):
                lsl = slice(T * c, T * (c + 1))
                z_tiles[c] = [None] * KT
                for half, wt in ((0, wix_t), (1, wiz_t)):
                    pi = [ps_in.tile([128, 2 * T], F32, tag=f"pi{j}", name=f"pi{j}")
                          for j in range(2)]
                    for kk in range(8):
                        h = hs_pool.tile([128, T], BF16, tag="hs")
                        nc.sync.dma_start(h[:], hsT[128 * kk:128 * (kk + 1), lsl])
                        for k in range(KT):
                            # 2 k-tiles share one psum bank as ONE accumulation
                            # group: start zeroes the whole bank, so only the
                            # first matmul starts and only the last stops
                            reg = pi[k // 2][:, T * (k % 2):T * (k % 2 + 1)]
                            nc.tensor.matmul(reg,
                                             wt[kk][:, 128 * k:128 * (k + 1)],
                                             h[:],
                                             start=(kk == 0 and k % 2 == 0),
                                             stop=(kk == 7 and k % 2 == 1))
                    for k in range(KT):
                        reg = pi[k // 2][:, T * (k % 2):T * (k % 2 + 1)]
                        if half == 0:
                            nc.scalar.copy(
                                xpad[k][:, 1 + T * c:1 + T * (c + 1)], reg)
                        else:
                            # zt = (tanh(z/2)+1)*z = 2*silu(z)
                            th = cvp.tile([128, T], F32, tag="zth")
                            nc.scalar.activation(th[:], reg, AF.Tanh, scale=0.5)
                            zt = z_p.tile([128, T], F16, tag=f"z{k}",
                                          name=f"z{k}")
                            z_tiles[c][k] = zt
                            _eng(nc, cfg["zt"]).scalar_tensor_tensor(
                                zt[:], th[:], 1.0, reg, OP.add, OP.mult)

            xs_tiles = {}

            def head(c):
                # ---- conv; xs = 2*silu(conv+bcv) (wx prescaled by 0.5) ----
                xs_c = xs_tiles[c] = []
                for k in range(KT):
                    base = T * c

                    def wcb(i):
                        col = wc_t[:, DC * k + i:DC * k + i + 1]
                        return col.to_broadcast([128, T])

                    if cfg["conv"] == "v":
                        t0 = cvp.tile([128, T], F32, tag="cv")
                        nc.vector.tensor_scalar(
                            t0[:], xpad[k][:, base:base + T],
                            wc_t[:, DC * k:DC * k + 1], None, OP.mult)
                        t3 = t0
                        for i in (1, 2, 3):
                            ti = cvp.tile([128, T], F32, tag="cv")
                            nc.vector.scalar_tensor_tensor(
                                ti[:], xpad[k][:, base + i:base + i + T],
                                wc_t[:, DC * k + i:DC * k + i + 1], t3[:],
                                OP.mult, OP.add)
                            t3 = ti
                    else:
                        # gpsimd has no tensor_scalar/STT: TT with free-dim
                        # broadcast weights, pairwise sum
                        m0 = cvp.tile([128, T], F32, tag="cv")
                        nc.gpsimd.tensor_tensor(m0[:], xpad[k][:, base:base + T],
                                                wcb(0), OP.mult)
                        m1 = cvp.tile([128, T], F32, tag="cv")
                        nc.gpsimd.tensor_tensor(
                            m1[:], xpad[k][:, base + 1:base + 1 + T], wcb(1),
                            OP.mult)
                        nc.gpsimd.tensor_tensor(m0[:], m0[:], m1[:], OP.add)
                        m2 = cvp.tile([128, T], F32, tag="cv2")
                        nc.gpsimd.tensor_tensor(
                            m2[:], xpad[k][:, base + 2:base + 2 + T], wcb(2),
                            OP.mult)
                        m3 = cvp.tile([128, T], F32, tag="cv2")
                        nc.gpsimd.tensor_tensor(
                            m3[:], xpad[k][:, base + 3:base + 3 + T], wcb(3),
                            OP.mult)
                        nc.gpsimd.tensor_tensor(m2[:], m2[:], m3[:], OP.add)
                        t3 = cvp.tile([128, T], F32, tag="cv3")
                        nc.gpsimd.tensor_tensor(t3[:], m0[:], m2[:], OP.add)
                    th = cvp.tile([128, T], F32, tag="cth")
                    nc.scalar.activation(th[:], t3[:], AF.Tanh, scale=0.5,
                                         bias=hbcv_t[:, k:k + 1])
                    xb = cvp.tile([128, T], F32, tag="xb")
                    bcvb = bcv_t[:, k:k + 1].to_broadcast([128, T])
                    ceng.tensor_tensor(xb[:], t3[:], bcvb, OP.add)
                    xk = xs_p.tile([128, T], F32R, tag=f"xs{k}", name=f"xs{k}")
                    if cfg["conv"] == "v":
                        nc.vector.scalar_tensor_tensor(xk[:], th[:], 1.0, xb[:],
                                                       OP.add, OP.mult)
                    else:
                        xm = cvp.tile([128, T], F32, tag="xm")
                        nc.gpsimd.tensor_tensor(xm[:], xb[:], th[:], OP.mult)
                        nc.vector.tensor_tensor(xk[:], xm[:], xb[:], OP.add)
                    xs_c.append(xk)

                # ---- x_dbl partial + AllReduce ----
                pxd = ps_x.tile([128, T], F32, tag="pxd", name="pxd")
                for k in range(KT):
                    nc.tensor.matmul(pxd[0:96, :], wxr_t[k][:], xs_c[k][:],
                                     start=(k == 0), stop=(k == KT - 1))
                xt = xd_p.tile([96, T], F16, tag="xt")
                nc.scalar.copy(xt[:], pxd[0:96, :])
                nc.sync.dma_start(xd_bounce[c][:], xt[:])
                if use_collective:
                    nc.gpsimd.collective_compute(
                        "AllReduce", OP.add,
                        replica_groups=[[0, 1, 2, 3], [4, 5, 6, 7]],
                        ins=[xd_bounce[c].opt()], outs=[xd_red[c].opt()])
                else:
                    nc.sync.dma_start(xd_red[c][:], xd_bounce[c][:])

            def stail(c):
                lsl = slice(T * c, T * (c + 1))
                xs_c = xs_tiles[c]
                xdb = xd_p.tile([64, T], F16, tag="xdb")
                nc.sync.dma_start(xdb[:], xd_red[c][0:DR, :])
                bct16 = bct_p.tile([2 * DS, T], F16, tag="bct16")
                nc.sync.dma_start(bct16[:], xd_red[c][DR:DR + 2 * DS, :])

                # ---- delta = softplus(dt): blocked Exp x4 then Ln x4 ----
                pdts = []
                for kp in range(2):
                    pdt = ps_bc.tile([128, 2 * T], F32, tag="pb")
                    for j in range(2):
                        k = 2 * kp + j
                        nc.tensor.matmul(
                            pdt[:, T * j:T * (j + 1)],
                            wdt_t[0:DR, 128 * k:128 * (k + 1)],
                            xdb[:], start=(j == 0), stop=(j == 1))
                    pdts.append(pdt)
                deltas = []
                for k in range(KT):
                    dlt = del_p.tile([128, T], F32, tag=f"d{k}", name=f"d{k}")
                    nc.scalar.activation(dlt[:], pdts[k // 2][:, T * (k % 2):
                                                             T * (k % 2 + 1)],
                                         AF.Exp, bias=bdt_t[:, k:k + 1])
                    deltas.append(dlt)
                for k in range(KT):
                    nc.scalar.activation(deltas[k][:], deltas[k][:], AF.Ln,
                                         bias=1.0)

                # ---- B/C partition broadcast ----
                b_bc = bc_p.tile([128, DS * T], EF, tag="b_bc")
                c_bc = bc_p.tile([128, DS * T], EF, tag="c_bc")
                for half, dst, cpc in ((0, b_bc, cfg["bcopy"]),
                                       (1, c_bc, cfg["ccopy"])):
                    pcopy = _copy(nc, cpc)
                    for g in range(DS // 2):
                        pb = ps_bc.tile([128, 2 * T], F32, tag="pb")
                        for j in range(2):
                            n = 16 * half + 2 * g + j
                            nc.tensor.matmul(pb[:, T * j:T * (j + 1)],
                                             sel16[:, 128 * n:128 * (n + 1)],
                                             bct16[:], start=(j == 0),
                                             stop=(j == 1))
                        pcopy(dst[:, 2 * T * g:2 * T * (g + 1)], pb[:])

                # ---- per-k scan blocks ----
                g_list = []
                for k in range(KT):
                    nsl = slice(DS * k, DS * (k + 1))
                    d_ch = deltas[k]
                    dx = mid.tile([128, T], EF, tag="dx")
                    _eng(nc, cfg["dx"]).tensor_tensor(
                        dx[:], d_ch[:], xs_c[k][:].bitcast(F32), OP.mult)
                    dA = mid.tile([128, DS * T], EF, tag="dA")
                    mode = {"a": "act", "p": "pow"}[(cfg["dA"] * KT)[k]]
                    if mode == "act":
                        for n in range(DS):
                            nc.scalar.activation(
                                dA[:, T * n:T * (n + 1)], d_ch[:], AF.Exp,
                                scale=asc_t[:, DS * k + n:DS * k + n + 1])
                    else:
                        nc.scalar.activation(dA[:, 0:T], d_ch[:], AF.Exp,
                                             scale=-1.0)
                        nc.vector.tensor_tensor(dA[:, T:2 * T], dA[:, 0:T],
                                                dA[:, 0:T], OP.mult)
                        for (lo, n_seg) in ((2, 2), (4, 4), (8, 8)):
                            src = dA[:, (lo - 1) * T:lo * T]
                            nc.vector.tensor_tensor(
                                dA[:, lo * T:2 * lo * T].rearrange(
                                    "p (n l) -> p n l", n=n_seg),
                                dA[:, 0:lo * T].rearrange(
                                    "p (n l) -> p n l", n=n_seg),
                                src[:, None, :].to_broadcast([128, n_seg, T]),
                                OP.mult)
                    dBu = mid.tile([128, DS * T], EF, tag="dBu")
                    deng = _eng(nc, (cfg["dbu"] * KT)[k])
                    deng.tensor_tensor(
                        dBu[:].rearrange("p (n l) -> p n l", n=DS),
                        dx[:, None, :].to_broadcast([128, DS, T]),
                        b_bc[:].rearrange("p (n l) -> p n l", n=DS),
                        OP.mult)
                    # fold carried state into first column of each segment
                    feng = _eng(nc, cfg["fold"])
                    fx = mid.tile([128, DS], EF, tag="fx")
                    feng.tensor_tensor(fx[:], dA[:, 0:DS * T:T],
                                       state_t[:, nsl], OP.mult)
                    feng.tensor_tensor(dBu[:, 0:DS * T:T],
                                       dBu[:, 0:DS * T:T], fx[:], OP.add)
                    nc.gpsimd.memset(dA[:, 0:DS * T:T], 0.0)
                    s_t = s_pool.tile([128, DS * T], EF, tag="s")
                    seng = nc.vector  # hw: TensorTensorScan only exists on DVE
                    seng.tensor_tensor_scan(s_t[:], dA[:], dBu[:], 0.0,
                                            OP.mult, OP.add)
                    seng.tensor_copy(state_t[:, nsl], s_t[:, T - 1:DS * T:T])
                    # y = sum_n s*C: in-place mult, then halving tree
                    meng.tensor_tensor(s_t[:], s_t[:], c_bc[:], OP.mult)
                    for w in (8, 4, 2):
                        teng.tensor_tensor(
                            s_t[:, 0:w * T], s_t[:, 0:w * T],
                            s_t[:, w * T:2 * w * T], OP.add)
                    y_r = yg_p.tile([128, T], F32, tag="yr")
                    teng.tensor_tensor(y_r[:], s_t[:, 0:T], s_t[:, T:2 * T],
                                       OP.add)
                    # skip + gate
                    nc.vector.scalar_tensor_tensor(
                        y_r[:], xs_c[k][:].bitcast(F32), dpar_t[:, k:k + 1],
                        y_r[:], OP.mult, OP.add)
                    g_t = yg_p.tile([128, T], F16, tag=f"g{k}", name=f"g{k}")
                    _eng(nc, cfg["gate"]).tensor_tensor(
                        g_t[:], y_r[:], z_tiles[c][k][:], OP.mult)
                    g_list.append(g_t)

                # ---- out_proj (wout prescaled by 0.25) ----
                ocopy = _copy(nc, cfg["otcopy"])
                for h in range(T // 128):
                    msl = slice(128 * h, 128 * (h + 1))
                    rsl = slice(T * c + 128 * h, T * c + 128 * (h + 1))
                    ot = outc.tile([128, DM], F16, tag="ot")
                    for col in range(2):
                        po = ps_out.tile([128, 512], F32, tag="po")
                        for k in range(KT):
                            nc.tensor.matmul(
                                po[:], g_list[k][:, msl],
                                wout_t[k][:, 512 * col:512 * (col + 1)],
                                start=(k == 0), stop=(k == KT - 1))
                        ocopy(ot[:, 512 * col:512 * (col + 1)], po[:])
                    nc.sync.dma_start(outp[rsl, :], ot[:])

            for c in range(NCH):
                in_proj(c)
                if c >= 1:
                    head(c - 1)
                if c >= 2:
                    stail(c - 2)
            head(NCH - 1)
            stail(NCH - 2)
            stail(NCH - 1)

        for _rep in range(n_reps):
            with ExitStack() as ctx:
                emit_once(ctx)
    nc.compile()
    return nc


def make_in_maps(inputs):
    hs = np.ascontiguousarray(np.asarray(inputs["hidden_states"], np.float32))
    W_in = np.asarray(inputs["W_in"], np.float32)
    W_conv = np.asarray(inputs["W_conv"], np.float32)
    b_conv = np.asarray(inputs["b_conv"], np.float32)
    W_x = np.asarray(inputs["W_x"], np.float32)
    W_dt = np.asarray(inputs["W_dt"], np.float32)
    b_dt = np.asarray(inputs["b_dt"], np.float32)
    A_log = np.asarray(inputs["A_log"], np.float32)
    D_param = np.asarray(inputs["D_param"], np.float32)
    W_out = np.asarray(inputs["W_out"], np.float32)
    A = -np.exp(A_log.astype(np.float64)).astype(np.float32)
    sel_mat = np.zeros((2 * DS, 2 * DS * 128), np.float16)
    for n in range(2 * DS):
        sel_mat[n, 128 * n:128 * (n + 1)] = 1.0

    in_maps = []
    for cid in range(8):
        b, s = cid // NSH, cid % NSH
        sh = slice(DL * s, DL * (s + 1))
        in_maps.append({
            "hsT": np.ascontiguousarray(hs[b].T).astype(ml_dtypes.bfloat16),
            "wix": np.ascontiguousarray(
                W_in[:, 2 * DL * s:2 * DL * (s + 1):2]).astype(ml_dtypes.bfloat16),
            "wiz": np.ascontiguousarray(
                W_in[:, 2 * DL * s + 1:2 * DL * (s + 1) + 1:2]).astype(
                    ml_dtypes.bfloat16),
            "wc": np.ascontiguousarray(W_conv[:, 0, sh].T),
            "bcv": np.ascontiguousarray(b_conv[sh].reshape(DL, 1)),
            # x is carried as 2*silu(.): fold 0.5 into W_x, 0.25 into W_out
            "wx": np.ascontiguousarray(0.5 * W_x[sh, :]),
            "wdt": np.ascontiguousarray(W_dt[:, sh].astype(np.float16)),
            "bdt": np.ascontiguousarray(b_dt[sh].reshape(DL, 1)),
            "asc": np.ascontiguousarray(A[sh, :]),
            "dpar": np.ascontiguousarray(D_param[sh].reshape(DL, 1)),
            "wout": np.ascontiguousarray(
                (0.25 * W_out[sh, :]).astype(np.float16)),
            "sel": sel_mat,
        })
    return in_maps


_NC_CACHE = None
_LAST_IN_MAPS = None


def kernel(**inputs) -> np.ndarray:
    global _NC_CACHE, _LAST_IN_MAPS
    in_maps = make_in_maps(inputs)
    _LAST_IN_MAPS = in_maps
    if _NC_CACHE is None:
        _NC_CACHE = build_program()
    res = run_bass_kernel_spmd(_NC_CACHE, in_maps, list(range(8)))
    out = np.zeros((B, L, DM), np.float32)
    for cid in range(8):
        out[cid // NSH] += res.results[cid]["outp"].astype(np.float32)
    return out


if __name__ == "__main__":
    rng = np.random.default_rng(0)
    dummy = {
        "hidden_states": rng.standard_normal((B, L, DM), dtype=np.float32),
        "W_in": rng.standard_normal((DM, 2 * DI), dtype=np.float32) * 0.03,
        "W_conv": rng.standard_normal((DC, 1, DI), dtype=np.float32) * 0.5,
        "b_conv": np.zeros((DI,), np.float32),
        "W_x": rng.standard_normal((DI, DR + 2 * DS), dtype=np.float32) * 0.02,
        "W_dt": rng.standard_normal((DR, DI), dtype=np.float32) * 0.12,
        "b_dt": rng.standard_normal((DI,), dtype=np.float32) * 0.01,
        "A_log": np.log(np.broadcast_to(np.arange(1, DS + 1, dtype=np.float32),
                                        (DI, DS))).copy(),
        "D_param": np.ones((DI,), np.float32),
        "W_out": rng.standard_normal((DI, DM), dtype=np.float32) * 0.03,
    }
    out = kernel(**dummy)
    print("out", out.shape, out.dtype, np.abs(out).max())
